# revision 7
# baseline (speedup 1.0000x reference)
"""Trainium2 Bass kernel for the ExemplarBaseline retrieval-kNN model.

Math (per batch b, fully independent across b):
    f      = data.reshape(B*T, CHW) @ W_fe + b_fe            (feature extract)
    d2     = ||f_s - f_t||^2 ; dist = d2**0.25
    sims   = exp(-c * dist)
    numers = 1e-8 + sum_{s<t} sims[s,t] * teach[s, cls]
    score  = numers**gamma / sum_cls ; score[t=0] = 1e-8

Sharding: data-parallel over the batch dim B (128) across 8 NeuronCores,
16 sequences per core (BL=16, T=128 -> TOK=2048 tokens per core).

Structure (v3):
  - Host pre-permutes x/W/teach so every DMA descriptor is a 1.5KB+
    contiguous run; ALL inputs (x 48KB/partition fp8 + W 24KB) live in
    SBUF, DMA'd up front in ~96-192KB pieces ordered by DEADLINE
    (x chunk0, W dt0, pars, W dt1..7, teach, x1..) so the round-robin
    queue assignment drains the critical ~1.5MB first.  Late x chunks
    are triggered from the ACT engine (second HWDGE) to halve the
    serial trigger-issuance on the sync engine.
  - feats^T = W^T x^T in fp8+DoubleRow (2x PE rate), evacuated with
    bias-add directly to fp8 fT pair tiles [128, 2, TOK].
  - sq via onesT @ (fT*fT) as fp8 DoubleRow (full-width ones stationary:
    the ISA check rejects narrow DR stationaries; psum row 0 is read).
  - Gram + rank-1 (-0.5*sq_s -0.5*sq_t) all fp8 DoubleRow into one PSUM
    group; rank-1 is an fp8 hi/lo pair of -0.5*sq/128 against a 128.0
    row (e4m3 tops out at 448 so sq~1024 can't be stored directly).
  - token chunks [384, 512, 512, 512, 128]: the small first chunk
    starts the PE sooner; the 128-token last chunk leaves only ONE
    sequence's epilogue after the final feats matmul (the tail was
    ~15us with a chunk-lagged uniform pipeline).
  - epilogue per SEQUENCE, split epiA (Gram+rank1+ACT chain launch) /
    epiB (numers+norm+output) and spread through the NEXT chunk's
    d-tile slots two slots apart, so the PE never waits on ACT chains.
  - sims in single bf16 (affine_select writes bf16 directly): one
    numers matmul instead of an exact hi/lo triple; teach in bf16.
  - scores PE-transposed (identity matmul) so each output DMA is
    10x512B descriptors instead of 128x40B.
All transcendentals use only Ln/Exp (one ACT table set, no reloads).
Error budget: rel err ~1.3e-2 measured vs the 2e-2 gate (fp8 feats
dominates; fp8 Gram/sq + bf16 sims/teach/score add the rest).
"""

import numpy as np
import ml_dtypes

B, T, NC = 128, 128, 10
CHW, D = 3072, 1024
NCORES = 8
BL = B // NCORES          # 16 sequences per core
TOK = BL * T              # 2048 tokens per core
KT = CHW // 128           # 24 contraction tiles
DT = D // 128             # 8 feature tiles

CHUNKS = [384, 512, 512, 512, 128]     # token columns per chunk
NSEQ = [w // T for w in CHUNKS]        # sequences per chunk [3,4,4,4,1]
C0 = [sum(CHUNKS[:i]) for i in range(len(CHUNKS))]   # chunk col starts

EPS_NUMER = 1e-8
SQSCALE = 128.0           # rank-1 fp8 scaling: store -0.5*sq/SQSCALE

_NC_CACHE = {}
LAST_RESULTS = None       # BassKernelResults of the most recent run (for test.py)


def _build_bass():
    import concourse.mybir as mybir
    import concourse.tile as tile
    from concourse import bacc

    f32 = mybir.dt.float32
    bf16 = mybir.dt.bfloat16
    fp8 = mybir.dt.float8e4
    AF = mybir.ActivationFunctionType
    OP = mybir.AluOpType
    PM = mybir.MatmulPerfMode

    # The ACT table-set chooser picks the FIRST set containing each function:
    # Exp -> set 0, Ln -> set 5, which makes every Ln<->Exp transition reload
    # tables (~1.3us each).  Both live together in natural_log_exp_and_others;
    # hide them from every other set so the chooser lands there once.
    if not getattr(bacc, "_ln_exp_tables_patched", False):
        orig_tables = bacc.get_activation_tables

        def _patched_tables(arch):
            out = {}
            for name, funcs in orig_tables(arch).items():
                if name != "natural_log_exp_and_others":
                    funcs = funcs - {AF.Ln, AF.Exp}
                out[name] = funcs
            return out

        bacc.get_activation_tables = _patched_tables
        bacc._ln_exp_tables_patched = True

    nc = bacc.Bacc("TRN2", target_bir_lowering=False)

    # Host-side layouts (see make_in_maps): per-chunk x tensors and
    # dt-major W so every DMA slice is contiguous per partition.
    x_h = [
        nc.dram_tensor(f"xh{c}", [128, KT * w], fp8, kind="ExternalInput")
        for c, w in enumerate(CHUNKS)
    ]
    W_h = nc.dram_tensor("Wh", [128, DT * KT * 128], fp8, kind="ExternalInput")
    teach_h = nc.dram_tensor("teach", [T, BL * NC], bf16, kind="ExternalInput")
    pars_h = nc.dram_tensor("pars", [128, 2 + DT], f32, kind="ExternalInput")
    ident_h = nc.dram_tensor("ident", [128, 128], bf16, kind="ExternalInput")
    y_h = nc.dram_tensor("yT", [BL, NC, T], f32, kind="ExternalOutput")

    with tile.TileContext(nc) as tc:
        with (
            tc.tile_pool(name="cpool", bufs=1) as cpool,
            tc.tile_pool(name="f2pool", bufs=2) as f2pool,
            tc.tile_pool(name="wpool", bufs=4) as wpool,
            tc.tile_pool(name="smpool", bufs=3) as smpool,
            tc.tile_pool(name="spool", bufs=6) as spool,
            tc.tile_pool(name="stpool", bufs=2) as stpool,
            tc.tile_pool(name="pfpool", bufs=2, space="PSUM") as pfpool,
            tc.tile_pool(name="psqpool", bufs=1, space="PSUM") as psqpool,
            tc.tile_pool(name="pgpool", bufs=2, space="PSUM") as pgpool,
            tc.tile_pool(name="pnpool", bufs=1, space="PSUM") as pnpool,
            tc.tile_pool(name="ptpool", bufs=1, space="PSUM") as ptpool,
        ):
            # ---- persistent tiles -------------------------------------
            W_sb = cpool.tile([128, DT, KT, 128], fp8, name="W_sb")
            x_sb = [
                cpool.tile([128, KT, w], fp8, name=f"x_sb{c}")
                for c, w in enumerate(CHUNKS)
            ]
            teach_sb = cpool.tile([128, BL, NC], bf16, name="teach_sb")
            pars_sb = cpool.tile([128, 2 + DT], f32, name="pars_sb")
            ident_sb = cpool.tile([128, 128], bf16, name="ident_sb")
            eps_sb = cpool.tile([128, 1], f32, name="eps_sb")
            # fT in fp8 DoubleRow pair layout: tile p holds d-tiles 2p, 2p+1
            fTp = [
                cpool.tile([128, 2, TOK], fp8, name=f"fTp{i}")
                for i in range(DT // 2)
            ]
            # full-width ones stationary: DoubleRow's ISA check rejects
            # narrow (<128) stationaries, so psq is [128, w] with every
            # partition holding the same sq row (row 0 is read).
            onesq = cpool.tile([128, 2, 128], fp8, name="onesq")
            # hi/lo fp8 pair of -0.5*sq/SQSCALE against a SQSCALE row
            sqp = cpool.tile([1, 2, TOK], fp8, name="sqp")
            orow = cpool.tile([1, 2, TOK], fp8, name="orow")
            negc = pars_sb[:, 0:1]
            gam = pars_sb[:, 1:2]

            # ---- all input DMAs, deadline order -----------------------
            # ~96-192KB pieces; round-robin queue assignment then drains
            # the critical x0+Wdt0 ~1.5MB first.  Late x chunks go on the
            # ACT engine's HWDGE to halve sync-side trigger issuance.
            KW = KT * 128

            def xpiece(eng, c, k0, k1):
                eng.dma_start(
                    out=x_sb[c][:, k0:k1, :],
                    in_=x_h[c][:, k0 * CHUNKS[c]:k1 * CHUNKS[c]],
                )

            def wpiece(eng, dt_i, k0, k1):
                eng.dma_start(
                    out=W_sb[:, dt_i, k0:k1, :],
                    in_=W_h[:, dt_i * KW + k0 * 128:dt_i * KW + k1 * 128],
                )

            for k in range(0, KT, 2):                  # x0: 12 x 96KB
                xpiece(nc.sync, 0, k, k + 2)
            for k in range(0, KT, 8):                  # W dt0: 3 x 128KB
                wpiece(nc.sync, 0, k, k + 8)
            nc.sync.dma_start(out=pars_sb, in_=pars_h[:, :])
            for dt_i in range(1, DT):                  # W dt1..7: 3 x 128KB
                for k in range(0, KT, 8):
                    wpiece(nc.sync, dt_i, k, k + 8)
            nc.sync.dma_start(out=teach_sb, in_=teach_h[:, :])
            for k in range(0, KT, 6):                  # x1,x2: 4 x 192KB each
                xpiece(nc.scalar, 1, k, k + 6)
            for k in range(0, KT, 6):
                xpiece(nc.scalar, 2, k, k + 6)
            nc.scalar.dma_start(out=ident_sb, in_=ident_h[:, :])
            for k in range(0, KT, 6):                  # x3: 4 x 192KB
                xpiece(nc.scalar, 3, k, k + 6)
            for k in range(0, KT, 12):                 # x4: 2 x 96KB
                xpiece(nc.scalar, 4, k, k + 12)

            # constants: single-partition orow is slow on DVE (~3.5us),
            # gpsimd is idle at startup
            nc.gpsimd.memset(onesq, 1.0)
            nc.gpsimd.memset(orow, SQSCALE)
            nc.vector.memset(eps_sb, EPS_NUMER)

            # ---- per-(chunk, d-tile) feats + fused sq -----------------
            state = {}

            def feats_dt(c, dt_i):
                w = CHUNKS[c]
                csl = slice(C0[c], C0[c] + w)
                pf = pfpool.tile([128, w], f32, name="pf")
                for k in range(0, KT, 2):
                    nc.tensor.matmul(
                        pf, W_sb[:, dt_i, k:k + 2, :], x_sb[c][:, k:k + 2, :],
                        start=(k == 0), stop=(k == KT - 2),
                        perf_mode=PM.DoubleRow,
                    )
                # evacuate psum -> fp8 fT pair tile with per-partition bias
                # add.  On DVE so the scalar engine only ever runs Ln/Exp.
                fsl = fTp[dt_i // 2][:, dt_i % 2, csl]
                nc.vector.tensor_scalar(
                    fsl, pf, pars_sb[:, 2 + dt_i:3 + dt_i], None, op0=OP.add,
                )
                if dt_i % 2 == 0:
                    state["f2p"] = f2pool.tile([128, 2, w], fp8, name="f2")
                if dt_i == 0:
                    state["psq"] = psqpool.tile([128, w], f32, name="psq")
                f2p = state["f2p"]
                nc.vector.tensor_mul(f2p[:, dt_i % 2, :], fsl, fsl)
                if dt_i % 2 == 1:
                    # sq accumulated over d on the PE: ones pair^T @ f2
                    nc.tensor.matmul(
                        state["psq"], onesq, f2p,
                        start=(dt_i == 1), stop=(dt_i == DT - 1),
                        perf_mode=PM.DoubleRow,
                    )

            def sqn_chain(c):
                w = CHUNKS[c]
                csl = slice(C0[c], C0[c] + w)
                sqf = spool.tile([1, w], f32, name="sqf")
                nc.vector.tensor_scalar(
                    sqf, state["psq"][0:1, :], -0.5 / SQSCALE, None,
                    op0=OP.mult,
                )
                nc.vector.tensor_copy(sqp[0:1, 0, csl], sqf)            # hi
                nc.vector.tensor_sub(sqp[0:1, 1, csl], sqf,
                                     sqp[0:1, 0, csl])                  # lo

            # ---- per-sequence epilogue, split A/B for pipelining ------
            sims_of = {}

            def epiA(b):
                tsl = slice(b * T, (b + 1) * T)
                # psum = G - 0.5*sq_s - 0.5*sq_t = -0.5 * d2
                pg = pgpool.tile([128, 128], f32, name="pg")
                for p in range(DT // 2):
                    nc.tensor.matmul(
                        pg, fTp[p][:, :, tsl], fTp[p][:, :, tsl],
                        start=(p == 0), stop=False, perf_mode=PM.DoubleRow,
                    )
                nc.tensor.matmul(
                    pg, sqp[:, :, tsl], orow[:, :, tsl],
                    start=False, stop=False, perf_mode=PM.DoubleRow,
                )
                nc.tensor.matmul(
                    pg, orow[:, :, tsl], sqp[:, :, tsl],
                    start=False, stop=True, perf_mode=PM.DoubleRow,
                )
                # dist = exp(0.25*ln(-2*psum)) = d2**0.25 straight off PSUM;
                # sims = exp(-c*dist).  Only the (masked-out) diagonal can
                # go NaN -- off-diagonal d2 ~ 2000 > 0.
                lt = wpool.tile([128, 128], f32, name="lt")
                nc.scalar.activation(lt, pg, AF.Ln, scale=-2.0)
                dist = wpool.tile([128, 128], f32, name="dist")
                nc.scalar.activation(dist, lt, AF.Exp, scale=0.25)
                sims = wpool.tile([128, 128], f32, name="sims")
                nc.scalar.activation(sims, dist, AF.Exp, scale=negc)
                # zero s >= t (kills diagonal NaNs too); writes bf16 for the
                # numers matmul.  iota = t - s - 1 >= 0 keeps s < t.
                simsM = smpool.tile([128, 128], bf16, name="simsM")
                nc.gpsimd.affine_select(
                    out=simsM, in_=sims,
                    compare_op=OP.is_ge, fill=0.0,
                    base=-1, pattern=[[1, 128]], channel_multiplier=-1,
                )
                sims_of[b] = simsM

            def epiB(b):
                # numers[t, cls] = sum_s simsM[s,t] * teach[s, cls]
                pn = pnpool.tile([128, NC], f32, name="pn")
                nc.tensor.matmul(
                    pn, sims_of.pop(b), teach_sb[:, b, :],
                    start=True, stop=True,
                )
                # tmp = (numers + eps) ** gamma  via exp(gamma * ln(.))
                l2 = spool.tile([128, NC], f32, name="l2")
                nc.scalar.activation(l2, pn, AF.Ln, bias=eps_sb)
                tmp = spool.tile([128, NC], f32, name="tmp")
                nc.scalar.activation(tmp, l2, AF.Exp, scale=gam)
                den = spool.tile([128, 1], f32, name="den")
                nc.vector.tensor_reduce(
                    den, tmp, axis=mybir.AxisListType.X, op=OP.add,
                )
                rden = spool.tile([128, 1], f32, name="rden")
                nc.vector.reciprocal(rden, den)
                scb = spool.tile([128, NC], bf16, name="scb")
                nc.vector.tensor_scalar(scb, tmp, rden, None, op0=OP.mult)
                pt = ptpool.tile([NC, 128], bf16, name="pt")
                nc.tensor.matmul(pt, scb, ident_sb, is_transpose=True,
                                 start=True, stop=True)
                scT = stpool.tile([NC, 128], f32, name="scT")
                nc.vector.tensor_copy(scT, pt)
                nc.vector.memset(scT[:, 0:1], EPS_NUMER)       # t == 0 col
                nc.sync.dma_start(out=y_h[b], in_=scT)

            # ---- schedule: epilogues of chunk c-1 spread through the
            # d-tile slots of chunk c; epiB 2 slots after its epiA so the
            # numers MM never waits on the ACT chain.
            seq0 = [sum(NSEQ[:i]) for i in range(len(CHUNKS))]
            for c in range(len(CHUNKS)):
                for dt_i in range(DT):
                    feats_dt(c, dt_i)
                    if c > 0:
                        b0, n = seq0[c - 1], NSEQ[c - 1]
                        if 1 <= dt_i <= n:
                            epiA(b0 + dt_i - 1)
                        if 3 <= dt_i <= n + 2:
                            epiB(b0 + dt_i - 3)
                sqn_chain(c)
            b0, n = seq0[-1], NSEQ[-1]
            order = [("A", i) for i in range(min(2, n))]
            for i in range(n):
                if i + 2 < n:
                    order.append(("A", i + 2))
                order.append(("B", i))
            for kind, i in order:
                (epiA if kind == "A" else epiB)(b0 + i)

    nc.compile()
    return nc


def _get_bass():
    if "nc" not in _NC_CACHE:
        _NC_CACHE["nc"] = _build_bass()
    return _NC_CACHE["nc"]


def make_in_maps(data_t, teaching_signal_t, W_fe, b_fe, c, gamma):
    """Host-side prep: cast to fp8/bf16, permute for contiguous DMAs, shard."""
    import concourse.mybir as mybir
    mmdt = mybir.dt.np(mybir.dt.float8e4)
    x8 = np.asarray(data_t, np.float32).reshape(B * T, CHW).astype(mmdt)
    W8 = np.asarray(W_fe, np.float32).astype(mmdt)
    # W: [kt*128+p, dt*128+m] -> [p][dt][kt*128+m]
    Wh = np.ascontiguousarray(
        W8.reshape(KT, 128, DT, 128).transpose(1, 2, 0, 3)
    ).reshape(128, DT * KT * 128)
    bfe_pd = np.asarray(b_fe, np.float32).reshape(DT, 128).T     # [128, DT]
    cval = np.float32(np.asarray(c, np.float32).reshape(-1)[0])
    gval = np.float32(np.asarray(gamma, np.float32).reshape(-1)[0])
    pars = np.empty((128, 2 + DT), np.float32)
    pars[:, 0] = -cval
    pars[:, 1] = gval
    pars[:, 2:] = bfe_pd
    ident = np.eye(128, dtype=ml_dtypes.bfloat16)
    teach16 = np.asarray(teaching_signal_t, np.float32).astype(
        ml_dtypes.bfloat16)

    in_maps = []
    for core in range(NCORES):
        rows = slice(core * TOK, (core + 1) * TOK)
        xt = x8[rows].T                               # [CHW, TOK]
        m = dict(Wh=Wh, pars=pars, ident=ident)
        for ci, w in enumerate(CHUNKS):
            # x^T[kt*128+p, C0+j] -> [p][kt*w+j]
            xc = xt[:, C0[ci]:C0[ci] + w].reshape(KT, 128, w)
            m[f"xh{ci}"] = np.ascontiguousarray(
                xc.transpose(1, 0, 2)).reshape(128, KT * w)
        m["teach"] = np.ascontiguousarray(
            teach16[core * BL:(core + 1) * BL].transpose(1, 0, 2)
        ).reshape(T, BL * NC)
        in_maps.append(m)
    return in_maps


def kernel(responses_t, data_t, teaching_signal_t, W_fe, b_fe, c, gamma):
    global LAST_RESULTS
    from concourse.bass_utils import run_bass_kernel_spmd

    in_maps = make_in_maps(data_t, teaching_signal_t, W_fe, b_fe, c, gamma)
    nc = _get_bass()
    res = run_bass_kernel_spmd(nc, in_maps, core_ids=list(range(NCORES)))
    LAST_RESULTS = res
    y = np.concatenate(
        [r["yT"].transpose(0, 2, 1) for r in res.results], axis=0)
    return np.ascontiguousarray(y[:, :, None, :].astype(np.float32))
